# revision 21
# baseline (speedup 1.0000x reference)
"""Trainium2 Bass kernel for the highway-ensemble module.

Math (per sample b):
    s_n    = clients_logit[n,b,:] @ ensemble_scale + ensemble_bias
    sig_n  = sigmoid(s_n)                    (> 0, so L1 norm == plain sum)
    wn_n   = sig_n / sum_m sig_m
    cal    = (sum_n wn_n * clients_logit[n,b,:]) * logit_scale + logit_bias
    carry  = sigmoid(mean_n(clients_feature[n,b,:]) @ W2 + b2)
    out    = carry * cal + (1 - carry) * new_logit[b,:]

Sharding: data-parallel over the batch dim B=8192 across 8 NeuronCores
(1024 rows each); the client dim N=8 stays local; tiny parameters are
replicated. Each core streams its shard once from HBM -> memory-bound.

Two compiled variants, dispatched at runtime on the parameter values:
  - general: arbitrary ensemble_scale / logit_scale / logit_bias
  - fast:    ensemble_scale==1, logit_scale==1, logit_bias==0 (the
             module's init values). Phase A degenerates to plain row
             sums, which run in the DVE's 2x single-input mode, and the
             calibration stage disappears.
"""

import sys

if "/opt/trn_rl_repo" not in sys.path:
    sys.path.insert(0, "/opt/trn_rl_repo")

from contextlib import ExitStack

import numpy as np

import concourse.bass as bass
import concourse.tile as tile
from concourse import bacc, mybir
from concourse import bass_utils
from concourse.bass_utils import run_bass_kernel_spmd

# Artifact upload targets a remote bucket that this container cannot reach;
# only used on trace runs.
bass_utils.upload_artifacts = lambda tmpdir: tmpdir

N_CORES = 8
N_CLIENTS = 8
B = 8192
C = 1000
F = 2048
PB = 128  # batch rows per SBUF tile (partition dim)

FP32 = mybir.dt.float32
ALU = mybir.AluOpType
ACTFN = mybir.ActivationFunctionType


def build_nc(b_shard: int = B // N_CORES, fast: bool = False):
    nc = bacc.Bacc(
        "TRN2", target_bir_lowering=False, debug=False, num_devices=N_CORES
    )
    cf = nc.dram_tensor(
        "clients_feature", [N_CLIENTS, b_shard, F], FP32, kind="ExternalInput"
    ).ap()
    cl = nc.dram_tensor(
        "clients_logit", [N_CLIENTS, b_shard, C], FP32, kind="ExternalInput"
    ).ap()
    nl = nc.dram_tensor("new_logit", [b_shard, C], FP32, kind="ExternalInput").ap()
    es = nc.dram_tensor("ensemble_scale", [C, 1], FP32, kind="ExternalInput").ap()
    eb = nc.dram_tensor("ensemble_bias", [1], FP32, kind="ExternalInput").ap()
    ls = nc.dram_tensor("logit_scale", [C], FP32, kind="ExternalInput").ap()
    lb = nc.dram_tensor("logit_bias", [C], FP32, kind="ExternalInput").ap()
    w2 = nc.dram_tensor("W2", [F, 1], FP32, kind="ExternalInput").ap()
    b2 = nc.dram_tensor("b2", [1], FP32, kind="ExternalInput").ap()
    out = nc.dram_tensor("out", [b_shard, C], FP32, kind="ExternalOutput").ap()

    ntiles = b_shard // PB

    with tile.TileContext(nc) as tc, ExitStack() as ctx:
        consts = ctx.enter_context(tc.tile_pool(name="consts", bufs=1))
        lp = ctx.enter_context(tc.tile_pool(name="lp", bufs=16 if fast else 12))
        fp = ctx.enter_context(tc.tile_pool(name="fp", bufs=8))
        np_ = ctx.enter_context(tc.tile_pool(name="np", bufs=2))
        op = ctx.enter_context(tc.tile_pool(name="op", bufs=2))
        fsp = ctx.enter_context(tc.tile_pool(name="fsp", bufs=1))
        scrp = ctx.enter_context(tc.tile_pool(name="scrp", bufs=1))
        wk = ctx.enter_context(tc.tile_pool(name="wk", bufs=2))
        sm = ctx.enter_context(tc.tile_pool(name="sm", bufs=3))

        # Parameters broadcast to all 128 partitions once.
        def bcast(src, cols, tag):
            t = consts.tile([PB, cols], FP32, tag=tag)
            nc.gpsimd.dma_start(out=t, in_=src.unsqueeze(0).to_broadcast([PB, cols]))
            return t

        if not fast:
            esB = bcast(es[:, 0], C, "esB")
            lsB = bcast(ls, C, "lsB")
            lbB = bcast(lb, C, "lbB")
        w2B = bcast(w2[:, 0], F, "w2B")
        ebB = bcast(eb, 1, "ebB")
        b2B = bcast(b2, 1, "b2B")

        prev = None  # state carried to the next iteration (1-tile pipeline)
        for it in range(ntiles + 1):
            if it < ntiles:
                b0 = it * PB

                # --- features: per-client partial dots feat_n . W2 ---------
                fscr = fsp.tile([PB, F], FP32, tag="fscr")
                dcols = sm.tile([PB, N_CLIENTS], FP32, tag="dcols")
                for n in range(N_CLIENTS):
                    ft = fp.tile([PB, F], FP32, tag="ft")
                    nc.scalar.dma_start(out=ft, in_=cf[n, b0 : b0 + PB, :])
                    nc.vector.scalar_tensor_tensor(
                        out=fscr,
                        in0=ft,
                        scalar=1.0,
                        in1=w2B,
                        op0=ALU.mult,
                        op1=ALU.mult,
                        accum_out=dcols[:, n : n + 1],
                    )

                # --- logit loads + phase A scores --------------------------
                Ls = []
                s = sm.tile([PB, N_CLIENTS], FP32, tag="s")
                scr = scrp.tile([PB, C], FP32, tag="scr")
                for n in range(N_CLIENTS):
                    L = lp.tile([PB, C], FP32, tag="L")
                    nc.sync.dma_start(out=L, in_=cl[n, b0 : b0 + PB, :])
                    Ls.append(L)
                    if fast:
                        # ensemble_scale == 1: s_n is a plain row sum on ACT.
                        nc.scalar.activation(
                            out=scr,
                            in_=L,
                            func=ACTFN.Identity,
                            bias=0.0,
                            scale=1.0,
                            accum_out=s[:, n : n + 1],
                        )
                    else:
                        nc.vector.scalar_tensor_tensor(
                            out=scr,
                            in0=L,
                            scalar=1.0,
                            in1=esB,
                            op0=ALU.mult,
                            op1=ALU.mult,
                            accum_out=s[:, n : n + 1],
                        )

                # sig = sigmoid(s + eb); rs = 1 / sum_n sig
                sig = sm.tile([PB, N_CLIENTS], FP32, tag="sig")
                nc.scalar.activation(
                    out=sig, in_=s, func=ACTFN.Sigmoid, bias=ebB[:, 0:1], scale=1.0
                )
                ssum = sm.tile([PB, 1], FP32, tag="ssum")
                nc.vector.tensor_reduce(
                    out=ssum, in_=sig, axis=mybir.AxisListType.X, op=ALU.add
                )
                rs = sm.tile([PB, 1], FP32, tag="rs")
                nc.vector.reciprocal(out=rs, in_=ssum)

                # --- weighted logit sum (unnormalized) ---------------------
                wl = wk.tile([PB, C], FP32, tag="wl")
                nc.vector.tensor_scalar_mul(out=wl, in0=Ls[0], scalar1=sig[:, 0:1])
                for n in range(1, N_CLIENTS):
                    nc.vector.scalar_tensor_tensor(
                        out=wl,
                        in0=Ls[n],
                        scalar=sig[:, n : n + 1],
                        in1=wl,
                        op0=ALU.mult,
                        op1=ALU.add,
                    )

                newt = np_.tile([PB, C], FP32, tag="new")
                nc.sync.dma_start(out=newt, in_=nl[b0 : b0 + PB, :])
                cur = dict(b0=b0, dcols=dcols, wl=wl, rs=rs, newt=newt)
            else:
                cur = None

            # --- deferred tail of the PREVIOUS tile (keeps next tile's
            # phase-A ACTIVATEs from queueing behind carry on ACT) ---------
            if prev is not None:
                dot = sm.tile([PB, 1], FP32, tag="dot")
                nc.vector.tensor_reduce(
                    out=dot, in_=prev["dcols"], axis=mybir.AxisListType.X, op=ALU.add
                )
                carry = sm.tile([PB, 1], FP32, tag="carry")
                nc.scalar.activation(
                    out=carry,
                    in_=dot,
                    func=ACTFN.Sigmoid,
                    bias=b2B[:, 0:1],
                    scale=1.0 / N_CLIENTS,
                )
                d = wk.tile([PB, C], FP32, tag="d")
                if fast:
                    # logit_scale == 1, logit_bias == 0: cal = wl * rs.
                    nc.vector.scalar_tensor_tensor(
                        out=d, in0=prev["wl"], scalar=prev["rs"][:, 0:1],
                        in1=prev["newt"], op0=ALU.mult, op1=ALU.subtract,
                    )
                else:
                    nc.vector.scalar_tensor_tensor(
                        out=d, in0=prev["wl"], scalar=prev["rs"][:, 0:1],
                        in1=lsB, op0=ALU.mult, op1=ALU.mult,
                    )
                    nc.vector.tensor_add(out=d, in0=d, in1=lbB)
                    nc.vector.tensor_sub(out=d, in0=d, in1=prev["newt"])
                o = op.tile([PB, C], FP32, tag="o")
                # out = (cal - new) * carry + new
                nc.vector.scalar_tensor_tensor(
                    out=o,
                    in0=d,
                    scalar=carry[:, 0:1],
                    in1=prev["newt"],
                    op0=ALU.mult,
                    op1=ALU.add,
                )
                nc.sync.dma_start(
                    out=out[prev["b0"] : prev["b0"] + PB, :], in_=o
                )
            prev = cur

    nc.compile()
    return nc


_NC_CACHE = {}


def _get_nc(b_shard, fast):
    key = (b_shard, fast)
    if key not in _NC_CACHE:
        _NC_CACHE[key] = build_nc(b_shard, fast)
    return _NC_CACHE[key]


def _run(inputs, trace=False, force_general=False):
    b = int(np.asarray(inputs["new_logit"]).shape[0])
    b_shard = b // N_CORES

    cf = np.ascontiguousarray(np.asarray(inputs["clients_feature"], dtype=np.float32))
    cl = np.ascontiguousarray(np.asarray(inputs["clients_logit"], dtype=np.float32))
    nl = np.ascontiguousarray(np.asarray(inputs["new_logit"], dtype=np.float32))
    rep = {
        k: np.ascontiguousarray(np.asarray(inputs[k], dtype=np.float32))
        for k in (
            "ensemble_scale",
            "ensemble_bias",
            "logit_scale",
            "logit_bias",
            "W2",
            "b2",
        )
    }

    fast = (
        not force_general
        and bool(np.all(rep["ensemble_scale"] == 1.0))
        and bool(np.all(rep["logit_scale"] == 1.0))
        and bool(np.all(rep["logit_bias"] == 0.0))
    )
    nc = _get_nc(b_shard, fast)

    in_maps = []
    for c in range(N_CORES):
        lo, hi = c * b_shard, (c + 1) * b_shard
        in_maps.append(
            {
                "clients_feature": np.ascontiguousarray(cf[:, lo:hi, :]),
                "clients_logit": np.ascontiguousarray(cl[:, lo:hi, :]),
                "new_logit": np.ascontiguousarray(nl[lo:hi, :]),
                **rep,
            }
        )

    res = run_bass_kernel_spmd(
        nc, in_maps, core_ids=list(range(N_CORES)), trace=trace
    )
    out = np.concatenate([res.results[c]["out"] for c in range(N_CORES)], axis=0)
    return out, res


def kernel(**inputs) -> np.ndarray:
    out, _ = _run(inputs, trace=False)
    return out


def kernel_traced(**inputs):
    """Like kernel() but returns (output, BassKernelResults) with NTFF timing."""
    return _run(inputs, trace=True)


def kernel_traced_general(**inputs):
    """Force the general (non-specialized) variant, traced."""
    return _run(inputs, trace=True, force_general=True)


# revision 22
# speedup vs baseline: 1.0473x; 1.0473x over previous
"""Trainium2 Bass kernel for the highway-ensemble module.

Math (per sample b):
    s_n    = clients_logit[n,b,:] @ ensemble_scale + ensemble_bias
    sig_n  = sigmoid(s_n)                    (> 0, so L1 norm == plain sum)
    wn_n   = sig_n / sum_m sig_m
    cal    = (sum_n wn_n * clients_logit[n,b,:]) * logit_scale + logit_bias
    carry  = sigmoid(mean_n(clients_feature[n,b,:]) @ W2 + b2)
    out    = carry * cal + (1 - carry) * new_logit[b,:]

Sharding: data-parallel over the batch dim B=8192 across 8 NeuronCores
(1024 rows each); the client dim N=8 stays local; tiny parameters are
replicated. Each core streams its shard once from HBM -> memory-bound.

Two compiled variants, dispatched at runtime on the parameter values:
  - general: arbitrary ensemble_scale / logit_scale / logit_bias
  - fast:    ensemble_scale==1, logit_scale==1, logit_bias==0 (the
             module's init values). Phase A degenerates to plain row
             sums, which run in the DVE's 2x single-input mode, and the
             calibration stage disappears.
"""

import sys

if "/opt/trn_rl_repo" not in sys.path:
    sys.path.insert(0, "/opt/trn_rl_repo")

from contextlib import ExitStack

import numpy as np

import concourse.bass as bass
import concourse.tile as tile
from concourse import bacc, mybir
from concourse import bass_utils
from concourse.bass_utils import run_bass_kernel_spmd

# Artifact upload targets a remote bucket that this container cannot reach;
# only used on trace runs.
bass_utils.upload_artifacts = lambda tmpdir: tmpdir

N_CORES = 8
N_CLIENTS = 8
B = 8192
C = 1000
F = 2048
PB = 128  # batch rows per SBUF tile (partition dim)

FP32 = mybir.dt.float32
ALU = mybir.AluOpType
ACTFN = mybir.ActivationFunctionType


def build_nc(b_shard: int = B // N_CORES, fast: bool = False):
    nc = bacc.Bacc(
        "TRN2", target_bir_lowering=False, debug=False, num_devices=N_CORES
    )
    cf = nc.dram_tensor(
        "clients_feature", [N_CLIENTS, b_shard, F], FP32, kind="ExternalInput"
    ).ap()
    cl = nc.dram_tensor(
        "clients_logit", [N_CLIENTS, b_shard, C], FP32, kind="ExternalInput"
    ).ap()
    nl = nc.dram_tensor("new_logit", [b_shard, C], FP32, kind="ExternalInput").ap()
    es = nc.dram_tensor("ensemble_scale", [C, 1], FP32, kind="ExternalInput").ap()
    eb = nc.dram_tensor("ensemble_bias", [1], FP32, kind="ExternalInput").ap()
    ls = nc.dram_tensor("logit_scale", [C], FP32, kind="ExternalInput").ap()
    lb = nc.dram_tensor("logit_bias", [C], FP32, kind="ExternalInput").ap()
    w2 = nc.dram_tensor("W2", [F, 1], FP32, kind="ExternalInput").ap()
    b2 = nc.dram_tensor("b2", [1], FP32, kind="ExternalInput").ap()
    out = nc.dram_tensor("out", [b_shard, C], FP32, kind="ExternalOutput").ap()

    ntiles = b_shard // PB

    with tile.TileContext(nc) as tc, ExitStack() as ctx:
        consts = ctx.enter_context(tc.tile_pool(name="consts", bufs=1))
        lp = ctx.enter_context(tc.tile_pool(name="lp", bufs=20 if fast else 16))
        fp = ctx.enter_context(tc.tile_pool(name="fp", bufs=5))
        np_ = ctx.enter_context(tc.tile_pool(name="np", bufs=2))
        op = ctx.enter_context(tc.tile_pool(name="op", bufs=2))
        fsp = ctx.enter_context(tc.tile_pool(name="fsp", bufs=2))
        scrp = ctx.enter_context(tc.tile_pool(name="scrp", bufs=1))
        wk = ctx.enter_context(tc.tile_pool(name="wk", bufs=2))
        sm = ctx.enter_context(tc.tile_pool(name="sm", bufs=3))

        # Parameters broadcast to all 128 partitions once.
        def bcast(src, cols, tag):
            t = consts.tile([PB, cols], FP32, tag=tag)
            nc.gpsimd.dma_start(out=t, in_=src.unsqueeze(0).to_broadcast([PB, cols]))
            return t

        if not fast:
            esB = bcast(es[:, 0], C, "esB")
            lsB = bcast(ls, C, "lsB")
            lbB = bcast(lb, C, "lbB")
        w2B = bcast(w2[:, 0], F, "w2B")
        ebB = bcast(eb, 1, "ebB")
        b2B = bcast(b2, 1, "b2B")

        prev = None  # state carried to the next iteration (1-tile pipeline)
        for it in range(ntiles + 1):
            if it < ntiles:
                b0 = it * PB

                # --- logit loads + phase A scores --------------------------
                Ls = []
                s = sm.tile([PB, N_CLIENTS], FP32, tag="s")
                scr = scrp.tile([PB, C], FP32, tag="scr")
                for n in range(N_CLIENTS):
                    L = lp.tile([PB, C], FP32, tag="L")
                    nc.sync.dma_start(out=L, in_=cl[n, b0 : b0 + PB, :])
                    Ls.append(L)
                    if fast:
                        # ensemble_scale == 1: s_n is a plain row sum on ACT.
                        nc.scalar.activation(
                            out=scr,
                            in_=L,
                            func=ACTFN.Identity,
                            bias=0.0,
                            scale=1.0,
                            accum_out=s[:, n : n + 1],
                        )
                    else:
                        nc.vector.scalar_tensor_tensor(
                            out=scr,
                            in0=L,
                            scalar=1.0,
                            in1=esB,
                            op0=ALU.mult,
                            op1=ALU.mult,
                            accum_out=s[:, n : n + 1],
                        )

                # --- features: per-client partial dots feat_n . W2 ---------
                fscr = fsp.tile([PB, F], FP32, tag="fscr")
                dcols = sm.tile([PB, N_CLIENTS], FP32, tag="dcols")
                for n in range(N_CLIENTS):
                    ft = fp.tile([PB, F], FP32, tag="ft")
                    nc.sync.dma_start(out=ft, in_=cf[n, b0 : b0 + PB, :])
                    with tc.high_priority():
                        nc.vector.scalar_tensor_tensor(
                            out=fscr,
                            in0=ft,
                            scalar=1.0,
                            in1=w2B,
                            op0=ALU.mult,
                            op1=ALU.mult,
                            accum_out=dcols[:, n : n + 1],
                        )

                # sig = sigmoid(s + eb); rs = 1 / sum_n sig
                sig = sm.tile([PB, N_CLIENTS], FP32, tag="sig")
                nc.scalar.activation(
                    out=sig, in_=s, func=ACTFN.Sigmoid, bias=ebB[:, 0:1], scale=1.0
                )
                ssum = sm.tile([PB, 1], FP32, tag="ssum")
                nc.vector.tensor_reduce(
                    out=ssum, in_=sig, axis=mybir.AxisListType.X, op=ALU.add
                )
                rs = sm.tile([PB, 1], FP32, tag="rs")
                nc.vector.reciprocal(out=rs, in_=ssum)

                # --- weighted logit sum (unnormalized) ---------------------
                wl = wk.tile([PB, C], FP32, tag="wl")
                nc.vector.tensor_scalar_mul(out=wl, in0=Ls[0], scalar1=sig[:, 0:1])
                for n in range(1, N_CLIENTS):
                    nc.vector.scalar_tensor_tensor(
                        out=wl,
                        in0=Ls[n],
                        scalar=sig[:, n : n + 1],
                        in1=wl,
                        op0=ALU.mult,
                        op1=ALU.add,
                    )

                newt = np_.tile([PB, C], FP32, tag="new")
                nc.sync.dma_start(out=newt, in_=nl[b0 : b0 + PB, :])
                cur = dict(b0=b0, dcols=dcols, wl=wl, rs=rs, newt=newt)
            else:
                cur = None

            # --- deferred tail of the PREVIOUS tile (keeps next tile's
            # phase-A ACTIVATEs from queueing behind carry on ACT) ---------
            if prev is not None:
                dot = sm.tile([PB, 1], FP32, tag="dot")
                nc.vector.tensor_reduce(
                    out=dot, in_=prev["dcols"], axis=mybir.AxisListType.X, op=ALU.add
                )
                carry = sm.tile([PB, 1], FP32, tag="carry")
                nc.scalar.activation(
                    out=carry,
                    in_=dot,
                    func=ACTFN.Sigmoid,
                    bias=b2B[:, 0:1],
                    scale=1.0 / N_CLIENTS,
                )
                d = wk.tile([PB, C], FP32, tag="d")
                if fast:
                    # logit_scale == 1, logit_bias == 0: cal = wl * rs.
                    nc.vector.scalar_tensor_tensor(
                        out=d, in0=prev["wl"], scalar=prev["rs"][:, 0:1],
                        in1=prev["newt"], op0=ALU.mult, op1=ALU.subtract,
                    )
                else:
                    nc.vector.scalar_tensor_tensor(
                        out=d, in0=prev["wl"], scalar=prev["rs"][:, 0:1],
                        in1=lsB, op0=ALU.mult, op1=ALU.mult,
                    )
                    nc.vector.tensor_add(out=d, in0=d, in1=lbB)
                    nc.vector.tensor_sub(out=d, in0=d, in1=prev["newt"])
                o = op.tile([PB, C], FP32, tag="o")
                # out = (cal - new) * carry + new
                nc.vector.scalar_tensor_tensor(
                    out=o,
                    in0=d,
                    scalar=carry[:, 0:1],
                    in1=prev["newt"],
                    op0=ALU.mult,
                    op1=ALU.add,
                )
                nc.sync.dma_start(
                    out=out[prev["b0"] : prev["b0"] + PB, :], in_=o
                )
            prev = cur

    nc.compile()
    return nc


_NC_CACHE = {}


def _get_nc(b_shard, fast):
    key = (b_shard, fast)
    if key not in _NC_CACHE:
        _NC_CACHE[key] = build_nc(b_shard, fast)
    return _NC_CACHE[key]


def _run(inputs, trace=False, force_general=False):
    b = int(np.asarray(inputs["new_logit"]).shape[0])
    b_shard = b // N_CORES

    cf = np.ascontiguousarray(np.asarray(inputs["clients_feature"], dtype=np.float32))
    cl = np.ascontiguousarray(np.asarray(inputs["clients_logit"], dtype=np.float32))
    nl = np.ascontiguousarray(np.asarray(inputs["new_logit"], dtype=np.float32))
    rep = {
        k: np.ascontiguousarray(np.asarray(inputs[k], dtype=np.float32))
        for k in (
            "ensemble_scale",
            "ensemble_bias",
            "logit_scale",
            "logit_bias",
            "W2",
            "b2",
        )
    }

    fast = (
        not force_general
        and bool(np.all(rep["ensemble_scale"] == 1.0))
        and bool(np.all(rep["logit_scale"] == 1.0))
        and bool(np.all(rep["logit_bias"] == 0.0))
    )
    nc = _get_nc(b_shard, fast)

    in_maps = []
    for c in range(N_CORES):
        lo, hi = c * b_shard, (c + 1) * b_shard
        in_maps.append(
            {
                "clients_feature": np.ascontiguousarray(cf[:, lo:hi, :]),
                "clients_logit": np.ascontiguousarray(cl[:, lo:hi, :]),
                "new_logit": np.ascontiguousarray(nl[lo:hi, :]),
                **rep,
            }
        )

    res = run_bass_kernel_spmd(
        nc, in_maps, core_ids=list(range(N_CORES)), trace=trace
    )
    out = np.concatenate([res.results[c]["out"] for c in range(N_CORES)], axis=0)
    return out, res


def kernel(**inputs) -> np.ndarray:
    out, _ = _run(inputs, trace=False)
    return out


def kernel_traced(**inputs):
    """Like kernel() but returns (output, BassKernelResults) with NTFF timing."""
    return _run(inputs, trace=True)


def kernel_traced_general(**inputs):
    """Force the general (non-specialized) variant, traced."""
    return _run(inputs, trace=True, force_general=True)
